# revision 25
# baseline (speedup 1.0000x reference)
"""Trainium2 Bass kernel for causal multi-head attention (B=2, T=2048, C=1024, H=16).

Contract: kernel(**inputs) takes the FULL unsharded inputs
    x      [2, 2048, 1024] f32
    W_qkv  [1024, 3072] f32
    b_qkv  [3072] f32
    W_proj [1024, 1024] f32
    b_proj [1024] f32
and returns the FULL output [2, 2048, 1024] f32.

Sharding over 8 NeuronCores: core c handles batch b = c//4 and heads
4*(c%4) .. 4*(c%4)+3 (tensor parallel over heads, data parallel over batch).
Each core computes a partial projection output [2048, 1024]; the host sums
the 4 partials per batch and adds b_proj plus the (softmax-invariant)
v-bias contribution b_v @ W_proj.

Device-side dataflow (per core, all matmuls in float32r):
  qT/kT [256ch, 2048] <- W_qkv_shard.T-free matmuls vs host-transposed xT
  v_aug [2048, 4, 128] = [v_h | 64 ones-cols] per head
  per (q-chunk n of 512, head h):
    s^T tiles [k 128, q 512] via K=64 matmuls (2 heads packed via base partition)
    p^T = exp(s^T/8) on ACT (paired tiles -> [128,1024] ACTIVATEs), causal mask
    multiplied only on the 4 diagonal tiles
    out^T accum [128, 512]: rows 0:64 = att_out^T, rows 64:128 = denominator
    (replicated by the 64 ones columns); normalize with partition-shifted
    reciprocal + multiply
  proj: y[tok 128, 512] accum over 2 channel chunks from att^T tiles
"""
import os
import sys
import numpy as np

for _p in ("/opt/trn_rl_repo",):
    if os.path.isdir(_p) and _p not in sys.path:
        sys.path.append(_p)

import concourse.bass as bass
import concourse.bacc as bacc
import concourse.tile as tile
from concourse import mybir
from concourse import bass_utils

F32 = mybir.dt.float32
F32R = mybir.dt.float32r
AF = mybir.ActivationFunctionType

B, T, C = 2, 2048, 1024
H, D = 16, 64
N_CORES = 8
HPC = H // 4  # heads per core = 4
CPC = HPC * D  # channels per core = 256
TQ = 512  # q chunk
NQ = T // TQ  # 4
NT = T // 128  # 16 token tiles
NK = C // 128  # 8 contraction chunks for qkv


def build_nc():
    nc = bacc.Bacc("TRN2", target_bir_lowering=False, debug=False)
    xT = nc.dram_tensor("xT", [C, T], F32R, kind="ExternalInput").ap()
    wq = nc.dram_tensor("wq", [C, CPC], F32R, kind="ExternalInput").ap()
    wk = nc.dram_tensor("wk", [C, CPC], F32R, kind="ExternalInput").ap()
    wv = nc.dram_tensor("wv", [C, CPC], F32R, kind="ExternalInput").ap()
    wp = nc.dram_tensor("wp", [CPC, C], F32R, kind="ExternalInput").ap()
    bq = nc.dram_tensor("bq", [CPC], F32, kind="ExternalInput").ap()
    bk = nc.dram_tensor("bk", [CPC], F32, kind="ExternalInput").ap()
    ones4 = nc.dram_tensor("ones4", [128, HPC, 64], F32R, kind="ExternalInput").ap()
    masks = nc.dram_tensor("masks", [128, 4, 2, TQ], F32R, kind="ExternalInput").ap()
    yp = nc.dram_tensor("yp", [T, C], F32, kind="ExternalOutput").ap()

    with tile.TileContext(nc) as tc:
        consts = tc.alloc_tile_pool(name="consts", bufs=1)
        persist = tc.alloc_tile_pool(name="persist", bufs=1)
        xpool = tc.alloc_tile_pool(name="xpool", bufs=2)
        ppool = tc.alloc_tile_pool(name="ppool", bufs=4)
        rpool = tc.alloc_tile_pool(name="rpool", bufs=3)
        ypool = tc.alloc_tile_pool(name="ypool", bufs=4)
        ps_main = tc.alloc_tile_pool(name="ps_main", bufs=2, space="PSUM")
        ps_av = tc.alloc_tile_pool(name="ps_av", bufs=2, space="PSUM")
        ps_s = tc.alloc_tile_pool(name="ps_s", bufs=2, space="PSUM")

        # ---- constants (emitted just-in-time so PE starts early) ----
        # single rearranged DMA per tensor: partition p reads strided rows
        wq_sb = consts.tile([128, NK, CPC], F32R, tag="wq")
        wk_sb = consts.tile([128, NK, CPC], F32R, tag="wk")
        bq_sb = consts.tile([128, 2], F32, tag="bq")
        bk_sb = consts.tile([128, 2], F32, tag="bk")
        wv_sb = consts.tile([128, NK, CPC], F32R, tag="wv")
        wp_sb = consts.tile([128, 2, C], F32R, tag="wp")
        masks_sb = consts.tile([128, 4, 2, TQ], F32R, tag="masks")

        # ---- persistent activations ----
        qT_sb = persist.tile([128, 2, T], F32R, tag="qT")
        kT_sb = persist.tile([128, 2, T], F32R, tag="kT")
        vaug = persist.tile([128, NT, HPC, 128], F32R, tag="vaug")
        attn = persist.tile([128, 2, T], F32R, tag="attn")

        xt_tiles = {}

        def fetch_xt(n):
            xt = xpool.tile([128, NK, TQ], F32R, tag="xt")
            xt_tiles[n] = xt
            if n == 0:
                # split first loads across three engine DMA queues so the
                # first matmuls start ASAP
                for kk in range(NK):
                    nc.sync.dma_start(
                        out=xt[:, kk, :],
                        in_=xT[kk * 128:(kk + 1) * 128, 0:TQ],
                    )
                    nc.scalar.dma_start(
                        out=wq_sb[:, kk, :], in_=wq[kk * 128:(kk + 1) * 128, :])
                    nc.gpsimd.dma_start(
                        out=wk_sb[:, kk, :], in_=wk[kk * 128:(kk + 1) * 128, :])
                nc.scalar.dma_start(out=bq_sb, in_=bq.rearrange("(m p) -> p m", p=128))
                nc.gpsimd.dma_start(out=bk_sb, in_=bk.rearrange("(m p) -> p m", p=128))
            else:
                nc.sync.dma_start(
                    out=xt,
                    in_=xT[:, n * TQ:(n + 1) * TQ].rearrange(
                        "(kk p) t -> p kk t", p=128),
                )

        def emit_qkv(n):
            # QKV + v for token chunk n
            xt = xt_tiles[n]
            if n + 1 < NQ:
                fetch_xt(n + 1)
            for m in range(2):
                psq = ps_main.tile([128, TQ], F32, tag="ps")
                psk = ps_main.tile([128, TQ], F32, tag="ps")
                for kk in range(NK):
                    nc.tensor.matmul(
                        psq, wq_sb[:, kk, m * 128:(m + 1) * 128], xt[:, kk, :],
                        start=(kk == 0), stop=(kk == NK - 1),
                    )
                for kk in range(NK):
                    nc.tensor.matmul(
                        psk, wk_sb[:, kk, m * 128:(m + 1) * 128], xt[:, kk, :],
                        start=(kk == 0), stop=(kk == NK - 1),
                    )
                nc.vector.tensor_scalar_add(
                    qT_sb[:, m, n * TQ:(n + 1) * TQ], psq.bitcast(F32R),
                    bq_sb[:, m:m + 1],
                )
                nc.vector.tensor_scalar_add(
                    kT_sb[:, m, n * TQ:(n + 1) * TQ], psk.bitcast(F32R),
                    bk_sb[:, m:m + 1],
                )
            if n == 0:
                # late-emitted consts: needed only from the v/attention phases
                nc.sync.dma_start(
                    out=wv_sb, in_=wv.rearrange("(kk p) c -> p kk c", p=128))
                nc.sync.dma_start(out=masks_sb, in_=masks)
                for tt in range(NT):
                    nc.sync.dma_start(out=vaug[:, tt, :, 64:128], in_=ones4)
                nc.sync.dma_start(
                    out=wp_sb, in_=wp.rearrange("(m p) c -> p m c", p=128))
            for t in range(4):
                tt = 4 * n + t
                psv = ps_main.tile([128, TQ], F32, tag="ps")
                for kk in range(NK):
                    nc.tensor.matmul(
                        psv[:, 0:CPC], xt[:, kk, t * 128:(t + 1) * 128],
                        wv_sb[:, kk, :],
                        start=(kk == 0), stop=(kk == NK - 1),
                    )
                nc.vector.tensor_copy(
                    vaug[:, tt, :, 0:64],
                    psv[:, 0:CPC].rearrange("p (h d) -> p h d", h=HPC).bitcast(F32R),
                )

        def emit_attn(n):
            # attention for q-chunk n: head pairs (2*m2, 2*m2+1) jointly.
            # the two heads sit at base partitions 0 and 64 -> their K=64
            # score matmuls land in disjoint PE row groups; exp covers both
            # heads in one [128, 1024] ACTIVATE.
            for m2 in range(2):
                q0 = qT_sb[0:64, m2, n * TQ:(n + 1) * TQ]
                q1 = qT_sb[64:128, m2, n * TQ:(n + 1) * TQ]
                pav0 = ps_av.tile([128, TQ], F32, tag="pav")
                pav1 = ps_av.tile([128, TQ], F32, tag="pav")
                h0, h1 = 2 * m2, 2 * m2 + 1
                J = 4 * n + 4
                pTs = {}

                def emit_score(j):
                    pst = ps_s.tile([128, 2, TQ], F32, tag="pst")
                    nc.tensor.matmul(
                        pst[:, 0, :],
                        kT_sb[0:64, m2, j * 128:(j + 1) * 128], q0,
                        start=True, stop=True,
                    )
                    nc.tensor.matmul(
                        pst[:, 1, :],
                        kT_sb[64:128, m2, j * 128:(j + 1) * 128], q1,
                        start=True, stop=True,
                    )
                    pT = ppool.tile([128, 2, TQ], F32R, tag="pT")
                    nc.scalar.activation(pT, pst, AF.Exp, scale=0.125)
                    d = j - 4 * n
                    if d >= 0:  # diagonal tile: causal mask, both heads at once
                        nc.vector.tensor_mul(pT, pT, masks_sb[:, d, :, :])
                    pTs[j] = pT

                def emit_av(j):
                    pT = pTs.pop(j)
                    nc.tensor.matmul(
                        pav0, vaug[:, j, h0, :], pT[:, 0, :],
                        start=(j == 0), stop=(j == J - 1),
                    )
                    nc.tensor.matmul(
                        pav1, vaug[:, j, h1, :], pT[:, 1, :],
                        start=(j == 0), stop=(j == J - 1),
                    )

                # skew: AV(j) trails score(j+1) so the exp round-trip is
                # off the in-order PE stream's critical path
                emit_score(0)
                for j in range(1, J):
                    emit_score(j)
                    emit_av(j - 1)
                emit_av(J - 1)
                for hh, pav in ((0, pav0), (1, pav1)):
                    # partition-shifting PSUM->SBUF copy; the custom-DVE
                    # reciprocal needs partition-aligned SBUF in/out.
                    dcp = rpool.tile([64, TQ], F32, tag="dcp")
                    nc.vector.tensor_copy(dcp, pav[64:128, :])
                    rdenb = rpool.tile([64, TQ], F32, tag="rden")
                    nc.vector.reciprocal_approx_fast(rdenb, dcp)
                    nc.vector.tensor_mul(
                        attn[hh * 64:hh * 64 + 64, m2, n * TQ:(n + 1) * TQ],
                        pav[0:64, :].bitcast(F32R), rdenb.bitcast(F32R),
                    )

        def emit_proj(n):
            # projection for token tiles of chunk n; lhsT reused across o
            for t in range(4):
                tt = 4 * n + t
                psy0 = ps_main.tile([128, TQ], F32, tag="ps")
                psy1 = ps_main.tile([128, TQ], F32, tag="ps")
                for m2 in range(2):
                    for o, psy in ((0, psy0), (1, psy1)):
                        nc.tensor.matmul(
                            psy,
                            attn[:, m2, tt * 128:(tt + 1) * 128],
                            wp_sb[:, m2, o * TQ:(o + 1) * TQ],
                            start=(m2 == 0), stop=(m2 == 1),
                        )
                for o, psy in ((0, psy0), (1, psy1)):
                    ysb = ypool.tile([128, TQ], F32, tag="y")
                    nc.vector.tensor_copy(ysb, psy)
                    nc.sync.dma_start(
                        out=yp[tt * 128:(tt + 1) * 128, o * TQ:(o + 1) * TQ],
                        in_=ysb,
                    )

        # software-pipelined emission: PE stream never blocks on the
        # normalize->proj dependency because the next chunk's QKV fills it.
        fetch_xt(0)
        emit_qkv(0)
        emit_qkv(1)
        emit_attn(0)
        emit_qkv(2)
        emit_proj(0)
        emit_attn(1)
        emit_qkv(3)
        emit_proj(1)
        emit_attn(2)
        emit_proj(2)
        emit_attn(3)
        emit_proj(3)

        for pool in (ps_s, ps_av, ps_main, ypool, rpool, ppool, xpool, persist, consts):
            pool.release()
    nc.compile()
    return nc


_NC_CACHE = []


def _get_nc():
    if not _NC_CACHE:
        _NC_CACHE.append(build_nc())
    return _NC_CACHE[0]


def _make_masks():
    jj = np.arange(128)[:, None]
    ii = np.arange(TQ)[None, :]
    m = np.zeros((128, 4, 2, TQ), dtype=np.float32)
    for d in range(4):
        md = (ii >= 128 * d + jj).astype(np.float32)
        m[:, d, 0, :] = md
        m[:, d, 1, :] = md
    return m


def kernel_in_maps(inputs):
    x = np.asarray(inputs["x"], dtype=np.float32)
    W_qkv = np.asarray(inputs["W_qkv"], dtype=np.float32)
    b_qkv = np.asarray(inputs["b_qkv"], dtype=np.float32)
    W_proj = np.asarray(inputs["W_proj"], dtype=np.float32)
    masks = _make_masks()
    ones4 = np.ones((128, HPC, 64), dtype=np.float32)
    in_maps = []
    for c in range(N_CORES):
        b, hg = c // 4, c % 4
        sl = slice(hg * CPC, (hg + 1) * CPC)
        in_maps.append({
            "xT": np.ascontiguousarray(x[b].T),
            "wq": np.ascontiguousarray(W_qkv[:, 0 * C:1 * C][:, sl]),
            "wk": np.ascontiguousarray(W_qkv[:, 1 * C:2 * C][:, sl]),
            "wv": np.ascontiguousarray(W_qkv[:, 2 * C:3 * C][:, sl]),
            "wp": np.ascontiguousarray(W_proj[hg * CPC:(hg + 1) * CPC, :]),
            "bq": np.ascontiguousarray(b_qkv[0 * C:1 * C][sl]),
            "bk": np.ascontiguousarray(b_qkv[1 * C:2 * C][sl]),
            "ones4": ones4,
            "masks": masks,
        })
    return in_maps


def kernel(x, W_qkv, b_qkv, W_proj, b_proj):
    x = np.asarray(x, dtype=np.float32)
    W_qkv = np.asarray(W_qkv, dtype=np.float32)
    b_qkv = np.asarray(b_qkv, dtype=np.float32)
    W_proj = np.asarray(W_proj, dtype=np.float32)
    b_proj = np.asarray(b_proj, dtype=np.float32)

    nc = _get_nc()
    in_maps = kernel_in_maps({
        "x": x, "W_qkv": W_qkv, "b_qkv": b_qkv, "W_proj": W_proj,
    })

    res = bass_utils.run_bass_kernel_spmd(nc, in_maps, core_ids=list(range(N_CORES)))

    # host-side reduction: sum head-group partials per batch, add biases.
    # v-bias contribution: out_h += b_v (softmax rows sum to 1) -> y += b_v @ W_proj
    bv = b_qkv[2 * C:3 * C]
    bias_full = (bv.astype(np.float64) @ W_proj.astype(np.float64)
                 + b_proj.astype(np.float64))
    y = np.zeros((B, T, C), dtype=np.float64)
    for c in range(N_CORES):
        y[c // 4] += res.results[c]["yp"].astype(np.float64)
    y += bias_full[None, None, :]
    return y.astype(np.float32)


# revision 26
# speedup vs baseline: 1.0267x; 1.0267x over previous
"""Trainium2 Bass kernel for causal multi-head attention (B=2, T=2048, C=1024, H=16).

Contract: kernel(**inputs) takes the FULL unsharded inputs
    x      [2, 2048, 1024] f32
    W_qkv  [1024, 3072] f32
    b_qkv  [3072] f32
    W_proj [1024, 1024] f32
    b_proj [1024] f32
and returns the FULL output [2, 2048, 1024] f32.

Sharding over 8 NeuronCores: core c handles batch b = c//4 and heads
4*(c%4) .. 4*(c%4)+3 (tensor parallel over heads, data parallel over batch).
Each core computes a partial projection output [2048, 1024]; the host sums
the 4 partials per batch and adds b_proj plus the (softmax-invariant)
v-bias contribution b_v @ W_proj.

Device-side dataflow (per core, all matmuls in float32r):
  qT/kT [256ch, 2048] <- W_qkv_shard.T-free matmuls vs host-transposed xT
  v_aug [2048, 4, 128] = [v_h | 64 ones-cols] per head
  per (q-chunk n of 512, head h):
    s^T tiles [k 128, q 512] via K=64 matmuls (2 heads packed via base partition)
    p^T = exp(s^T/8) on ACT (paired tiles -> [128,1024] ACTIVATEs), causal mask
    multiplied only on the 4 diagonal tiles
    out^T accum [128, 512]: rows 0:64 = att_out^T, rows 64:128 = denominator
    (replicated by the 64 ones columns); normalize with partition-shifted
    reciprocal + multiply
  proj: y[tok 128, 512] accum over 2 channel chunks from att^T tiles
"""
import os
import sys
import numpy as np

for _p in ("/opt/trn_rl_repo",):
    if os.path.isdir(_p) and _p not in sys.path:
        sys.path.append(_p)

import concourse.bass as bass
import concourse.bacc as bacc
import concourse.tile as tile
from concourse import mybir
from concourse import bass_utils

F32 = mybir.dt.float32
F32R = mybir.dt.float32r
AF = mybir.ActivationFunctionType

B, T, C = 2, 2048, 1024
H, D = 16, 64
N_CORES = 8
HPC = H // 4  # heads per core = 4
CPC = HPC * D  # channels per core = 256
TQ = 512  # q chunk
NQ = T // TQ  # 4
NT = T // 128  # 16 token tiles
NK = C // 128  # 8 contraction chunks for qkv


def build_nc():
    nc = bacc.Bacc("TRN2", target_bir_lowering=False, debug=False)
    xT = nc.dram_tensor("xT", [C, T], F32R, kind="ExternalInput").ap()
    wq = nc.dram_tensor("wq", [C, CPC], F32R, kind="ExternalInput").ap()
    wk = nc.dram_tensor("wk", [C, CPC], F32R, kind="ExternalInput").ap()
    wv = nc.dram_tensor("wv", [C, CPC], F32R, kind="ExternalInput").ap()
    wp = nc.dram_tensor("wp", [CPC, C], F32R, kind="ExternalInput").ap()
    bq = nc.dram_tensor("bq", [CPC], F32, kind="ExternalInput").ap()
    bk = nc.dram_tensor("bk", [CPC], F32, kind="ExternalInput").ap()
    ones4 = nc.dram_tensor("ones4", [128, HPC, 64], F32R, kind="ExternalInput").ap()
    masks = nc.dram_tensor("masks", [128, 4, TQ], F32R, kind="ExternalInput").ap()
    yp = nc.dram_tensor("yp", [T, C], F32, kind="ExternalOutput").ap()

    with tile.TileContext(nc) as tc:
        consts = tc.alloc_tile_pool(name="consts", bufs=1)
        persist = tc.alloc_tile_pool(name="persist", bufs=1)
        xpool = tc.alloc_tile_pool(name="xpool", bufs=2)
        ppool = tc.alloc_tile_pool(name="ppool", bufs=4)
        rpool = tc.alloc_tile_pool(name="rpool", bufs=3)
        ypool = tc.alloc_tile_pool(name="ypool", bufs=4)
        ps_main = tc.alloc_tile_pool(name="ps_main", bufs=2, space="PSUM")
        ps_av = tc.alloc_tile_pool(name="ps_av", bufs=2, space="PSUM")
        ps_s = tc.alloc_tile_pool(name="ps_s", bufs=2, space="PSUM")

        # ---- constants (emitted just-in-time so PE starts early) ----
        # single rearranged DMA per tensor: partition p reads strided rows
        wq_sb = consts.tile([128, NK, CPC], F32R, tag="wq")
        wk_sb = consts.tile([128, NK, CPC], F32R, tag="wk")
        bq_sb = consts.tile([128, 2], F32, tag="bq")
        bk_sb = consts.tile([128, 2], F32, tag="bk")
        wv_sb = consts.tile([128, NK, CPC], F32R, tag="wv")
        wp_sb = consts.tile([128, 2, C], F32R, tag="wp")
        masks_sb = consts.tile([128, 4, TQ], F32R, tag="masks")

        # ---- persistent activations ----
        qT_sb = persist.tile([128, 2, T], F32R, tag="qT")
        kT_sb = persist.tile([128, 2, T], F32R, tag="kT")
        vaug = persist.tile([128, NT, HPC, 128], F32R, tag="vaug")
        attn = persist.tile([128, 2, T], F32R, tag="attn")

        xt_tiles = {}

        def fetch_xt(n):
            xt = xpool.tile([128, NK, TQ], F32R, tag="xt")
            xt_tiles[n] = xt
            if n == 0:
                # split first loads across three engine DMA queues so the
                # first matmuls start ASAP
                for kk in range(NK):
                    nc.sync.dma_start(
                        out=xt[:, kk, :],
                        in_=xT[kk * 128:(kk + 1) * 128, 0:TQ],
                    )
                    nc.scalar.dma_start(
                        out=wq_sb[:, kk, :], in_=wq[kk * 128:(kk + 1) * 128, :])
                    nc.gpsimd.dma_start(
                        out=wk_sb[:, kk, :], in_=wk[kk * 128:(kk + 1) * 128, :])
                nc.scalar.dma_start(out=bq_sb, in_=bq.rearrange("(m p) -> p m", p=128))
                nc.gpsimd.dma_start(out=bk_sb, in_=bk.rearrange("(m p) -> p m", p=128))
            else:
                nc.sync.dma_start(
                    out=xt,
                    in_=xT[:, n * TQ:(n + 1) * TQ].rearrange(
                        "(kk p) t -> p kk t", p=128),
                )

        def emit_qkv(n):
            # QKV + v for token chunk n
            xt = xt_tiles[n]
            if n + 1 < NQ:
                fetch_xt(n + 1)
            for m in range(2):
                psq = ps_main.tile([128, TQ], F32, tag="ps")
                psk = ps_main.tile([128, TQ], F32, tag="ps")
                for kk in range(NK):
                    nc.tensor.matmul(
                        psq, wq_sb[:, kk, m * 128:(m + 1) * 128], xt[:, kk, :],
                        start=(kk == 0), stop=(kk == NK - 1),
                    )
                for kk in range(NK):
                    nc.tensor.matmul(
                        psk, wk_sb[:, kk, m * 128:(m + 1) * 128], xt[:, kk, :],
                        start=(kk == 0), stop=(kk == NK - 1),
                    )
                nc.vector.tensor_scalar_add(
                    qT_sb[:, m, n * TQ:(n + 1) * TQ], psq.bitcast(F32R),
                    bq_sb[:, m:m + 1],
                )
                nc.vector.tensor_scalar_add(
                    kT_sb[:, m, n * TQ:(n + 1) * TQ], psk.bitcast(F32R),
                    bk_sb[:, m:m + 1],
                )
            if n == 0:
                # late-emitted consts: needed only from the v/attention phases
                nc.sync.dma_start(
                    out=wv_sb, in_=wv.rearrange("(kk p) c -> p kk c", p=128))
                nc.sync.dma_start(out=masks_sb, in_=masks)
                for tt in range(NT):
                    nc.sync.dma_start(out=vaug[:, tt, :, 64:128], in_=ones4)
                nc.sync.dma_start(
                    out=wp_sb, in_=wp.rearrange("(m p) c -> p m c", p=128))
            for t in range(4):
                tt = 4 * n + t
                psv = ps_main.tile([128, TQ], F32, tag="ps")
                for kk in range(NK):
                    nc.tensor.matmul(
                        psv[:, 0:CPC], xt[:, kk, t * 128:(t + 1) * 128],
                        wv_sb[:, kk, :],
                        start=(kk == 0), stop=(kk == NK - 1),
                    )
                nc.vector.tensor_copy(
                    vaug[:, tt, :, 0:64],
                    psv[:, 0:CPC].rearrange("p (h d) -> p h d", h=HPC).bitcast(F32R),
                )

        def emit_attn(n):
            # attention for q-chunk n: head pairs (2*m2, 2*m2+1) jointly.
            # the two heads sit at base partitions 0 and 64 -> their K=64
            # score matmuls land in disjoint PE row groups; exp covers both
            # heads in one [128, 1024] ACTIVATE.
            for m2 in range(2):
                q0 = qT_sb[0:64, m2, n * TQ:(n + 1) * TQ]
                q1 = qT_sb[64:128, m2, n * TQ:(n + 1) * TQ]
                pav0 = ps_av.tile([128, TQ], F32, tag="pav")
                pav1 = ps_av.tile([128, TQ], F32, tag="pav")
                h0, h1 = 2 * m2, 2 * m2 + 1
                J = 4 * n + 4
                pTs = {}

                def emit_score(j):
                    pst = ps_s.tile([128, 2, TQ], F32, tag="pst")
                    nc.tensor.matmul(
                        pst[:, 0, :],
                        kT_sb[0:64, m2, j * 128:(j + 1) * 128], q0,
                        start=True, stop=True,
                    )
                    nc.tensor.matmul(
                        pst[:, 1, :],
                        kT_sb[64:128, m2, j * 128:(j + 1) * 128], q1,
                        start=True, stop=True,
                    )
                    pT = ppool.tile([128, 2, TQ], F32R, tag="pT")
                    nc.scalar.activation(pT, pst, AF.Exp, scale=0.125)
                    d = j - 4 * n
                    if d >= 0:  # diagonal tile: causal mask on both heads
                        nc.vector.tensor_mul(pT[:, 0, :], pT[:, 0, :], masks_sb[:, d, :])
                        nc.vector.tensor_mul(pT[:, 1, :], pT[:, 1, :], masks_sb[:, d, :])
                    pTs[j] = pT

                def emit_av(j):
                    pT = pTs.pop(j)
                    nc.tensor.matmul(
                        pav0, vaug[:, j, h0, :], pT[:, 0, :],
                        start=(j == 0), stop=(j == J - 1),
                    )
                    nc.tensor.matmul(
                        pav1, vaug[:, j, h1, :], pT[:, 1, :],
                        start=(j == 0), stop=(j == J - 1),
                    )

                # skew: AV(j) trails score(j+1) so the exp round-trip is
                # off the in-order PE stream's critical path
                emit_score(0)
                for j in range(1, J):
                    emit_score(j)
                    emit_av(j - 1)
                emit_av(J - 1)
                for hh, pav in ((0, pav0), (1, pav1)):
                    # partition-shifting PSUM->SBUF copy; the custom-DVE
                    # reciprocal needs partition-aligned SBUF in/out.
                    dcp = rpool.tile([64, TQ], F32, tag="dcp")
                    nc.vector.tensor_copy(dcp, pav[64:128, :])
                    rdenb = rpool.tile([64, TQ], F32, tag="rden")
                    nc.vector.reciprocal_approx_fast(rdenb, dcp)
                    nc.vector.tensor_mul(
                        attn[hh * 64:hh * 64 + 64, m2, n * TQ:(n + 1) * TQ],
                        pav[0:64, :].bitcast(F32R), rdenb.bitcast(F32R),
                    )

        def emit_proj(n):
            # projection for token tiles of chunk n; lhsT reused across o
            for t in range(4):
                tt = 4 * n + t
                psy0 = ps_main.tile([128, TQ], F32, tag="ps")
                psy1 = ps_main.tile([128, TQ], F32, tag="ps")
                for m2 in range(2):
                    for o, psy in ((0, psy0), (1, psy1)):
                        nc.tensor.matmul(
                            psy,
                            attn[:, m2, tt * 128:(tt + 1) * 128],
                            wp_sb[:, m2, o * TQ:(o + 1) * TQ],
                            start=(m2 == 0), stop=(m2 == 1),
                        )
                for o, psy in ((0, psy0), (1, psy1)):
                    ysb = ypool.tile([128, TQ], F32, tag="y")
                    nc.vector.tensor_copy(ysb, psy)
                    nc.sync.dma_start(
                        out=yp[tt * 128:(tt + 1) * 128, o * TQ:(o + 1) * TQ],
                        in_=ysb,
                    )

        # software-pipelined emission: PE stream never blocks on the
        # normalize->proj dependency because the next chunk's QKV fills it.
        fetch_xt(0)
        emit_qkv(0)
        emit_qkv(1)
        emit_attn(0)
        emit_qkv(2)
        emit_proj(0)
        emit_attn(1)
        emit_qkv(3)
        emit_proj(1)
        emit_attn(2)
        emit_proj(2)
        emit_attn(3)
        emit_proj(3)

        for pool in (ps_s, ps_av, ps_main, ypool, rpool, ppool, xpool, persist, consts):
            pool.release()
    nc.compile()
    return nc


_NC_CACHE = []


def _get_nc():
    if not _NC_CACHE:
        _NC_CACHE.append(build_nc())
    return _NC_CACHE[0]


def _make_masks():
    jj = np.arange(128)[:, None]
    ii = np.arange(TQ)[None, :]
    m = np.zeros((128, 4, TQ), dtype=np.float32)
    for d in range(4):
        m[:, d, :] = (ii >= 128 * d + jj).astype(np.float32)
    return m


def kernel_in_maps(inputs):
    x = np.asarray(inputs["x"], dtype=np.float32)
    W_qkv = np.asarray(inputs["W_qkv"], dtype=np.float32)
    b_qkv = np.asarray(inputs["b_qkv"], dtype=np.float32)
    W_proj = np.asarray(inputs["W_proj"], dtype=np.float32)
    masks = _make_masks()
    ones4 = np.ones((128, HPC, 64), dtype=np.float32)
    in_maps = []
    for c in range(N_CORES):
        b, hg = c // 4, c % 4
        sl = slice(hg * CPC, (hg + 1) * CPC)
        in_maps.append({
            "xT": np.ascontiguousarray(x[b].T),
            "wq": np.ascontiguousarray(W_qkv[:, 0 * C:1 * C][:, sl]),
            "wk": np.ascontiguousarray(W_qkv[:, 1 * C:2 * C][:, sl]),
            "wv": np.ascontiguousarray(W_qkv[:, 2 * C:3 * C][:, sl]),
            "wp": np.ascontiguousarray(W_proj[hg * CPC:(hg + 1) * CPC, :]),
            "bq": np.ascontiguousarray(b_qkv[0 * C:1 * C][sl]),
            "bk": np.ascontiguousarray(b_qkv[1 * C:2 * C][sl]),
            "ones4": ones4,
            "masks": masks,
        })
    return in_maps


def kernel(x, W_qkv, b_qkv, W_proj, b_proj):
    x = np.asarray(x, dtype=np.float32)
    W_qkv = np.asarray(W_qkv, dtype=np.float32)
    b_qkv = np.asarray(b_qkv, dtype=np.float32)
    W_proj = np.asarray(W_proj, dtype=np.float32)
    b_proj = np.asarray(b_proj, dtype=np.float32)

    nc = _get_nc()
    in_maps = kernel_in_maps({
        "x": x, "W_qkv": W_qkv, "b_qkv": b_qkv, "W_proj": W_proj,
    })

    res = bass_utils.run_bass_kernel_spmd(nc, in_maps, core_ids=list(range(N_CORES)))

    # host-side reduction: sum head-group partials per batch, add biases.
    # v-bias contribution: out_h += b_v (softmax rows sum to 1) -> y += b_v @ W_proj
    bv = b_qkv[2 * C:3 * C]
    bias_full = (bv.astype(np.float64) @ W_proj.astype(np.float64)
                 + b_proj.astype(np.float64))
    y = np.zeros((B, T, C), dtype=np.float64)
    for c in range(N_CORES):
        y[c // 4] += res.results[c]["yp"].astype(np.float64)
    y += bias_full[None, None, :]
    return y.astype(np.float32)


# revision 27
# speedup vs baseline: 1.0357x; 1.0087x over previous
"""Trainium2 Bass kernel for causal multi-head attention (B=2, T=2048, C=1024, H=16).

Contract: kernel(**inputs) takes the FULL unsharded inputs
    x      [2, 2048, 1024] f32
    W_qkv  [1024, 3072] f32
    b_qkv  [3072] f32
    W_proj [1024, 1024] f32
    b_proj [1024] f32
and returns the FULL output [2, 2048, 1024] f32.

Sharding over 8 NeuronCores: core c handles batch b = c//4 and heads
4*(c%4) .. 4*(c%4)+3 (tensor parallel over heads, data parallel over batch).
Each core computes a partial projection output [2048, 1024]; the host sums
the 4 partials per batch and adds b_proj plus the (softmax-invariant)
v-bias contribution b_v @ W_proj.

Device-side dataflow (per core, all matmuls in float32r):
  qT/kT [256ch, 2048] <- W_qkv_shard.T-free matmuls vs host-transposed xT
  v_aug [2048, 4, 128] = [v_h | 64 ones-cols] per head
  per (q-chunk n of 512, head h):
    s^T tiles [k 128, q 512] via K=64 matmuls (2 heads packed via base partition)
    p^T = exp(s^T/8) on ACT (paired tiles -> [128,1024] ACTIVATEs), causal mask
    multiplied only on the 4 diagonal tiles
    out^T accum [128, 512]: rows 0:64 = att_out^T, rows 64:128 = denominator
    (replicated by the 64 ones columns); normalize with partition-shifted
    reciprocal + multiply
  proj: y[tok 128, 512] accum over 2 channel chunks from att^T tiles
"""
import os
import sys
import numpy as np

for _p in ("/opt/trn_rl_repo",):
    if os.path.isdir(_p) and _p not in sys.path:
        sys.path.append(_p)

import concourse.bass as bass
import concourse.bacc as bacc
import concourse.tile as tile
from concourse import mybir
from concourse import bass_utils

F32 = mybir.dt.float32
F32R = mybir.dt.float32r
AF = mybir.ActivationFunctionType

B, T, C = 2, 2048, 1024
H, D = 16, 64
N_CORES = 8
HPC = H // 4  # heads per core = 4
CPC = HPC * D  # channels per core = 256
TQ = 512  # q chunk
NQ = T // TQ  # 4
NT = T // 128  # 16 token tiles
NK = C // 128  # 8 contraction chunks for qkv


def build_nc():
    nc = bacc.Bacc("TRN2", target_bir_lowering=False, debug=False)
    xT = nc.dram_tensor("xT", [C, T], F32R, kind="ExternalInput").ap()
    wq = nc.dram_tensor("wq", [C, CPC], F32R, kind="ExternalInput").ap()
    wk = nc.dram_tensor("wk", [C, CPC], F32R, kind="ExternalInput").ap()
    wv = nc.dram_tensor("wv", [C, CPC], F32R, kind="ExternalInput").ap()
    wp = nc.dram_tensor("wp", [CPC, C], F32R, kind="ExternalInput").ap()
    bq = nc.dram_tensor("bq", [CPC], F32, kind="ExternalInput").ap()
    bk = nc.dram_tensor("bk", [CPC], F32, kind="ExternalInput").ap()
    ones4 = nc.dram_tensor("ones4", [128, HPC, 64], F32R, kind="ExternalInput").ap()
    masks = nc.dram_tensor("masks", [128, 4, TQ], F32R, kind="ExternalInput").ap()
    yp = nc.dram_tensor("yp", [T, C], F32, kind="ExternalOutput").ap()

    with tile.TileContext(nc) as tc:
        consts = tc.alloc_tile_pool(name="consts", bufs=1)
        persist = tc.alloc_tile_pool(name="persist", bufs=1)
        xpool = tc.alloc_tile_pool(name="xpool", bufs=2)
        ppool = tc.alloc_tile_pool(name="ppool", bufs=6)
        rpool = tc.alloc_tile_pool(name="rpool", bufs=4)
        ypool = tc.alloc_tile_pool(name="ypool", bufs=6)
        ps_main = tc.alloc_tile_pool(name="ps_main", bufs=2, space="PSUM")
        ps_av = tc.alloc_tile_pool(name="ps_av", bufs=2, space="PSUM")
        ps_s = tc.alloc_tile_pool(name="ps_s", bufs=2, space="PSUM")

        # ---- constants (emitted just-in-time so PE starts early) ----
        # single rearranged DMA per tensor: partition p reads strided rows
        wq_sb = consts.tile([128, NK, CPC], F32R, tag="wq")
        wk_sb = consts.tile([128, NK, CPC], F32R, tag="wk")
        bq_sb = consts.tile([128, 2], F32, tag="bq")
        bk_sb = consts.tile([128, 2], F32, tag="bk")
        wv_sb = consts.tile([128, NK, CPC], F32R, tag="wv")
        wp_sb = consts.tile([128, 2, C], F32R, tag="wp")
        masks_sb = consts.tile([128, 4, TQ], F32R, tag="masks")

        # ---- persistent activations ----
        qT_sb = persist.tile([128, 2, T], F32R, tag="qT")
        kT_sb = persist.tile([128, 2, T], F32R, tag="kT")
        vaug = persist.tile([128, NT, HPC, 128], F32R, tag="vaug")
        attn = persist.tile([128, 2, T], F32R, tag="attn")

        xt_tiles = {}

        def fetch_xt(n):
            xt = xpool.tile([128, NK, TQ], F32R, tag="xt")
            xt_tiles[n] = xt
            if n == 0:
                # split first loads across three engine DMA queues so the
                # first matmuls start ASAP
                for kk in range(NK):
                    nc.sync.dma_start(
                        out=xt[:, kk, :],
                        in_=xT[kk * 128:(kk + 1) * 128, 0:TQ],
                    )
                    nc.scalar.dma_start(
                        out=wq_sb[:, kk, :], in_=wq[kk * 128:(kk + 1) * 128, :])
                    nc.gpsimd.dma_start(
                        out=wk_sb[:, kk, :], in_=wk[kk * 128:(kk + 1) * 128, :])
                nc.scalar.dma_start(out=bq_sb, in_=bq.rearrange("(m p) -> p m", p=128))
                nc.gpsimd.dma_start(out=bk_sb, in_=bk.rearrange("(m p) -> p m", p=128))
            else:
                nc.sync.dma_start(
                    out=xt,
                    in_=xT[:, n * TQ:(n + 1) * TQ].rearrange(
                        "(kk p) t -> p kk t", p=128),
                )

        def emit_qkv(n):
            # QKV + v for token chunk n
            xt = xt_tiles[n]
            if n + 1 < NQ:
                fetch_xt(n + 1)
            for m in range(2):
                psq = ps_main.tile([128, TQ], F32, tag="ps")
                psk = ps_main.tile([128, TQ], F32, tag="ps")
                for kk in range(NK):
                    nc.tensor.matmul(
                        psq, wq_sb[:, kk, m * 128:(m + 1) * 128], xt[:, kk, :],
                        start=(kk == 0), stop=(kk == NK - 1),
                    )
                for kk in range(NK):
                    nc.tensor.matmul(
                        psk, wk_sb[:, kk, m * 128:(m + 1) * 128], xt[:, kk, :],
                        start=(kk == 0), stop=(kk == NK - 1),
                    )
                nc.vector.tensor_scalar_add(
                    qT_sb[:, m, n * TQ:(n + 1) * TQ], psq.bitcast(F32R),
                    bq_sb[:, m:m + 1],
                )
                nc.vector.tensor_scalar_add(
                    kT_sb[:, m, n * TQ:(n + 1) * TQ], psk.bitcast(F32R),
                    bk_sb[:, m:m + 1],
                )
            if n == 0:
                # late-emitted consts: needed only from the v/attention phases
                nc.sync.dma_start(
                    out=wv_sb, in_=wv.rearrange("(kk p) c -> p kk c", p=128))
                nc.sync.dma_start(out=masks_sb, in_=masks)
                for tt in range(NT):
                    nc.sync.dma_start(out=vaug[:, tt, :, 64:128], in_=ones4)
                nc.sync.dma_start(
                    out=wp_sb, in_=wp.rearrange("(m p) c -> p m c", p=128))
            for t in range(4):
                tt = 4 * n + t
                psv = ps_main.tile([128, TQ], F32, tag="ps")
                for kk in range(NK):
                    nc.tensor.matmul(
                        psv[:, 0:CPC], xt[:, kk, t * 128:(t + 1) * 128],
                        wv_sb[:, kk, :],
                        start=(kk == 0), stop=(kk == NK - 1),
                    )
                nc.vector.tensor_copy(
                    vaug[:, tt, :, 0:64],
                    psv[:, 0:CPC].rearrange("p (h d) -> p h d", h=HPC).bitcast(F32R),
                )

        def emit_attn(n):
            # attention for q-chunk n: head pairs (2*m2, 2*m2+1) jointly.
            # the two heads sit at base partitions 0 and 64 -> their K=64
            # score matmuls land in disjoint PE row groups; exp covers both
            # heads in one [128, 1024] ACTIVATE.
            for m2 in range(2):
                q0 = qT_sb[0:64, m2, n * TQ:(n + 1) * TQ]
                q1 = qT_sb[64:128, m2, n * TQ:(n + 1) * TQ]
                pav0 = ps_av.tile([128, TQ], F32, tag="pav")
                pav1 = ps_av.tile([128, TQ], F32, tag="pav")
                h0, h1 = 2 * m2, 2 * m2 + 1
                J = 4 * n + 4
                pTs = {}

                def emit_score(j):
                    pst = ps_s.tile([128, 2, TQ], F32, tag="pst")
                    nc.tensor.matmul(
                        pst[:, 0, :],
                        kT_sb[0:64, m2, j * 128:(j + 1) * 128], q0,
                        start=True, stop=True,
                    )
                    nc.tensor.matmul(
                        pst[:, 1, :],
                        kT_sb[64:128, m2, j * 128:(j + 1) * 128], q1,
                        start=True, stop=True,
                    )
                    pT = ppool.tile([128, 2, TQ], F32R, tag="pT")
                    nc.scalar.activation(pT, pst, AF.Exp, scale=0.125)
                    d = j - 4 * n
                    if d >= 0:  # diagonal tile: causal mask on both heads
                        nc.vector.tensor_mul(pT[:, 0, :], pT[:, 0, :], masks_sb[:, d, :])
                        nc.vector.tensor_mul(pT[:, 1, :], pT[:, 1, :], masks_sb[:, d, :])
                    pTs[j] = pT

                def emit_av(j):
                    pT = pTs.pop(j)
                    nc.tensor.matmul(
                        pav0, vaug[:, j, h0, :], pT[:, 0, :],
                        start=(j == 0), stop=(j == J - 1),
                    )
                    nc.tensor.matmul(
                        pav1, vaug[:, j, h1, :], pT[:, 1, :],
                        start=(j == 0), stop=(j == J - 1),
                    )

                # skew: AV(j) trails score(j+1) so the exp round-trip is
                # off the in-order PE stream's critical path
                emit_score(0)
                for j in range(1, J):
                    emit_score(j)
                    emit_av(j - 1)
                emit_av(J - 1)
                for hh, pav in ((0, pav0), (1, pav1)):
                    # partition-shifting PSUM->SBUF copy; the custom-DVE
                    # reciprocal needs partition-aligned SBUF in/out.
                    dcp = rpool.tile([64, TQ], F32, tag="dcp")
                    nc.vector.tensor_copy(dcp, pav[64:128, :])
                    rdenb = rpool.tile([64, TQ], F32, tag="rden")
                    nc.vector.reciprocal_approx_fast(rdenb, dcp)
                    nc.vector.tensor_mul(
                        attn[hh * 64:hh * 64 + 64, m2, n * TQ:(n + 1) * TQ],
                        pav[0:64, :].bitcast(F32R), rdenb.bitcast(F32R),
                    )

        def emit_proj(n):
            # projection for token tiles of chunk n; lhsT reused across o
            for t in range(4):
                tt = 4 * n + t
                psy0 = ps_main.tile([128, TQ], F32, tag="ps")
                psy1 = ps_main.tile([128, TQ], F32, tag="ps")
                for m2 in range(2):
                    for o, psy in ((0, psy0), (1, psy1)):
                        nc.tensor.matmul(
                            psy,
                            attn[:, m2, tt * 128:(tt + 1) * 128],
                            wp_sb[:, m2, o * TQ:(o + 1) * TQ],
                            start=(m2 == 0), stop=(m2 == 1),
                        )
                for o, psy in ((0, psy0), (1, psy1)):
                    ysb = ypool.tile([128, TQ], F32, tag="y")
                    nc.vector.tensor_copy(ysb, psy)
                    nc.sync.dma_start(
                        out=yp[tt * 128:(tt + 1) * 128, o * TQ:(o + 1) * TQ],
                        in_=ysb,
                    )

        # software-pipelined emission: PE stream never blocks on the
        # normalize->proj dependency because the next chunk's QKV fills it.
        fetch_xt(0)
        emit_qkv(0)
        emit_qkv(1)
        emit_attn(0)
        emit_qkv(2)
        emit_proj(0)
        emit_attn(1)
        emit_qkv(3)
        emit_proj(1)
        emit_attn(2)
        emit_proj(2)
        emit_attn(3)
        emit_proj(3)

        for pool in (ps_s, ps_av, ps_main, ypool, rpool, ppool, xpool, persist, consts):
            pool.release()
    nc.compile()
    return nc


_NC_CACHE = []


def _get_nc():
    if not _NC_CACHE:
        _NC_CACHE.append(build_nc())
    return _NC_CACHE[0]


def _make_masks():
    jj = np.arange(128)[:, None]
    ii = np.arange(TQ)[None, :]
    m = np.zeros((128, 4, TQ), dtype=np.float32)
    for d in range(4):
        m[:, d, :] = (ii >= 128 * d + jj).astype(np.float32)
    return m


def kernel_in_maps(inputs):
    x = np.asarray(inputs["x"], dtype=np.float32)
    W_qkv = np.asarray(inputs["W_qkv"], dtype=np.float32)
    b_qkv = np.asarray(inputs["b_qkv"], dtype=np.float32)
    W_proj = np.asarray(inputs["W_proj"], dtype=np.float32)
    masks = _make_masks()
    ones4 = np.ones((128, HPC, 64), dtype=np.float32)
    in_maps = []
    for c in range(N_CORES):
        b, hg = c // 4, c % 4
        sl = slice(hg * CPC, (hg + 1) * CPC)
        in_maps.append({
            "xT": np.ascontiguousarray(x[b].T),
            "wq": np.ascontiguousarray(W_qkv[:, 0 * C:1 * C][:, sl]),
            "wk": np.ascontiguousarray(W_qkv[:, 1 * C:2 * C][:, sl]),
            "wv": np.ascontiguousarray(W_qkv[:, 2 * C:3 * C][:, sl]),
            "wp": np.ascontiguousarray(W_proj[hg * CPC:(hg + 1) * CPC, :]),
            "bq": np.ascontiguousarray(b_qkv[0 * C:1 * C][sl]),
            "bk": np.ascontiguousarray(b_qkv[1 * C:2 * C][sl]),
            "ones4": ones4,
            "masks": masks,
        })
    return in_maps


def kernel(x, W_qkv, b_qkv, W_proj, b_proj):
    x = np.asarray(x, dtype=np.float32)
    W_qkv = np.asarray(W_qkv, dtype=np.float32)
    b_qkv = np.asarray(b_qkv, dtype=np.float32)
    W_proj = np.asarray(W_proj, dtype=np.float32)
    b_proj = np.asarray(b_proj, dtype=np.float32)

    nc = _get_nc()
    in_maps = kernel_in_maps({
        "x": x, "W_qkv": W_qkv, "b_qkv": b_qkv, "W_proj": W_proj,
    })

    res = bass_utils.run_bass_kernel_spmd(nc, in_maps, core_ids=list(range(N_CORES)))

    # host-side reduction: sum head-group partials per batch, add biases.
    # v-bias contribution: out_h += b_v (softmax rows sum to 1) -> y += b_v @ W_proj
    bv = b_qkv[2 * C:3 * C]
    bias_full = (bv.astype(np.float64) @ W_proj.astype(np.float64)
                 + b_proj.astype(np.float64))
    y = np.zeros((B, T, C), dtype=np.float64)
    for c in range(N_CORES):
        y[c // 4] += res.results[c]["yp"].astype(np.float64)
    y += bias_full[None, None, :]
    return y.astype(np.float32)
